# revision 2
# baseline (speedup 1.0000x reference)
"""GCN ConvBlock (GCNConv + LayerNorm) on 8 Trainium2 NeuronCores.

Math: out = LayerNorm(A_hat @ x @ W + b) * gamma + beta, with
A_hat = D^-1/2 (A + I) D^-1/2 over N=10000 nodes / E=640000 edges.

Strategy (dense blocked matmul, dst-sharded across 8 cores):
  - A_hat factors as diag(dinv) @ C @ diag(dinv), C[s,d] = edge counts + I.
    C is exact in fp8; host pre-scales x' = dinv*x and shards 1250 dst
    columns per core.  Each core accumulates aggT[f=128, dst=1250] in PSUM
    over all 10000 src rows, then scales by dinv[dst], applies W (bf16),
    + b, LayerNorm (bn_stats/bn_aggr), *gamma, +beta, and streams the
    [dst,128] result out on the ACT HWDGE ring.
  - VARIANT "nodr": 79 src blocks of K=128, x' bf16 stationary, C fp8
    moving (1 col/cycle).  W single bf16 pass (hi/lo split dropped: the
    residual is ~1e-4 of signal; measured end-to-end rel err 2.9e-3).
  - VARIANT "drsi"/"dr": x' split hi/lo into two fp8 planes (x = hi + lo
    exactly to ~0.4%), DoubleRow K=256 passes (2 MACs/cell/cycle), 40
    superblocks x 2 planes.  "drsi" uses host-interleaved weights
    (DoubleRowSwInterleave).
  - The LN/W tail of iteration i is interleaved into the A-matmul packs of
    iteration i+1 so the PE never stalls on the DVE za dependency.
"""

import numpy as np
import ml_dtypes

N = 10000
E = 640000
D = 128
EPS = 1e-5

VARIANT = "nodr"          # "nodr" | "dr" | "drsi"
PIPELINE_TAIL = True

NCORES = 8
DST_PER_CORE = 1250
DST_PAD = 1280
CHUNKS = [(0, 512), (512, 512), (1024, 226)]
T_ROWS = [128] * 9 + [98]

BF16 = ml_dtypes.bfloat16
FP8 = ml_dtypes.float8_e4m3

_nc_cache = {}


def build_nc(n_iter=1, variant=None):
    variant = variant or VARIANT
    key = (n_iter, variant)
    if key in _nc_cache:
        return _nc_cache[key]
    import concourse.tile as tile
    from concourse import bacc, mybir

    f32 = mybir.dt.float32
    bf16 = mybir.dt.bfloat16
    fp8 = mybir.dt.float8e4

    dr = variant in ("dr", "drsi")
    pm = None
    if dr:
        pm = (mybir.MatmulPerfMode.DoubleRowSwInterleave if variant == "drsi"
              else mybir.MatmulPerfMode.DoubleRow)
    nsb = 40 if dr else 79            # superblocks of 256 / blocks of 128
    nx = 2 if dr else 1               # hi/lo stationary planes per superblock
    xdt = fp8 if dr else bf16
    xw = nsb * nx * (256 if dr else 128)

    nc = bacc.Bacc("TRN2", target_bir_lowering=False, debug=False,
                   enable_asserts=False, num_devices=NCORES)

    xs_d = nc.dram_tensor("xs", [128, xw], xdt, kind="ExternalInput").ap()
    ab_d = nc.dram_tensor("ab", [128, nsb * (2 if dr else 1) * DST_PAD], fp8,
                          kind="ExternalInput").ap()
    wt_d = nc.dram_tensor("wt", [128, 128], bf16, kind="ExternalInput").ap()
    dv_d = nc.dram_tensor("dv", [128, DST_PER_CORE], f32, kind="ExternalInput").ap()
    bb_d = nc.dram_tensor("bb", [128, 128], f32, kind="ExternalInput").ap()
    gb_d = nc.dram_tensor("gb", [128, 128], f32, kind="ExternalInput").ap()
    be_d = nc.dram_tensor("be", [128, 128], f32, kind="ExternalInput").ap()
    out_d = nc.dram_tensor("out", [DST_PAD, 128], f32, kind="ExternalOutput").ap()

    with tile.TileContext(nc) as tc:
        with (
            tc.tile_pool(name="const", bufs=1) as cpool,
            tc.tile_pool(name="work", bufs=2) as wpool,
            tc.tile_pool(name="ln", bufs=4) as lpool,
            tc.tile_pool(name="psA", bufs=2, space="PSUM") as psA,
            tc.tile_pool(name="psO", bufs=2, space="PSUM") as psO,
        ):
            if dr:
                xs = cpool.tile([128, nsb * nx * 2, 128], xdt)
            else:
                xs = cpool.tile([128, xw], xdt)
            nc.sync.dma_start(xs, xs_d)
            wt = cpool.tile([128, 128], bf16)
            nc.scalar.dma_start(wt, wt_d)
            dv = cpool.tile([128, DST_PER_CORE], f32)
            nc.scalar.dma_start(dv, dv_d)
            bb = cpool.tile([128, 128], f32)
            nc.scalar.dma_start(bb, bb_d)
            gb = cpool.tile([128, 128], f32)
            nc.scalar.dma_start(gb, gb_d)
            be = cpool.tile([128, 128], f32)
            nc.scalar.dma_start(be, be_d)
            eps_t = cpool.tile([128, 1], f32)
            nc.vector.memset(eps_t, EPS)
            if dr:
                cfull = cpool.tile([128, nsb * 2, DST_PAD], fp8)
            else:
                cfull = cpool.tile([128, nsb * DST_PAD], fp8)

            packs = []
            rem = nsb
            while rem > 0:
                packs.append(min(8, rem))
                rem -= min(8, rem)
            npacks = len(packs)

            def emit_tail_tile(t, za):
                rows = T_ROWS[t]
                cw = min(128, DST_PER_CORE - t * 128)
                po = psO.tile([128, 128], f32, tag="po", name="po")
                nc.tensor.matmul(po[:rows, :], lhsT=za[:, t * 128:t * 128 + cw],
                                 rhs=wt[:, 0:128], start=True, stop=True)
                zb = lpool.tile([128, 128], f32, tag="zb", name="zb")
                nc.vector.tensor_add(zb[:rows], po[:rows, :], bb[:rows])
                st = lpool.tile([128, 6], f32, tag="st", name="st")
                nc.vector.bn_stats(st[:rows], zb[:rows])
                mv = lpool.tile([128, 2], f32, tag="mv", name="mv")
                nc.vector.bn_aggr(mv[:rows], st[:rows])
                rs = lpool.tile([128, 1], f32, tag="rs", name="rs")
                nc.scalar.activation(out=rs[:rows], in_=mv[:rows, 1:2],
                                     func=mybir.ActivationFunctionType.Sqrt,
                                     bias=eps_t[:rows], scale=1.0)
                nc.vector.reciprocal(rs[:rows], rs[:rows])
                zn = lpool.tile([128, 128], f32, tag="zn", name="zn")
                nc.vector.tensor_scalar(out=zn[:rows], in0=zb[:rows],
                                        scalar1=mv[:rows, 0:1], scalar2=rs[:rows],
                                        op0=mybir.AluOpType.subtract,
                                        op1=mybir.AluOpType.mult)
                nc.vector.tensor_mul(zn[:rows], zn[:rows], gb[:rows])
                nc.vector.tensor_add(zn[:rows], zn[:rows], be[:rows])
                nc.scalar.dma_start(out_d[t * 128:t * 128 + rows, :], zn[:rows])

            # tail tiles of iter i-1 interleaved after each pack of iter i
            tpp = (10 + npacks - 1) // npacks     # tail tiles per pack
            prev_za = None

            for _it in range(n_iter):
                ps = [psA.tile([128, sz], f32, tag=f"ps{ci}", name=f"ps{ci}")
                      for ci, (_off, sz) in enumerate(CHUNKS)]
                sb0 = 0
                for pk, npk in enumerate(packs):
                    if _it == 0:
                        if dr:
                            nc.sync.dma_start(
                                cfull[:, 2 * sb0:2 * (sb0 + npk), :],
                                ab_d[:, 2 * sb0 * DST_PAD:2 * (sb0 + npk) * DST_PAD])
                        else:
                            nc.sync.dma_start(
                                cfull[:, sb0 * DST_PAD:(sb0 + npk) * DST_PAD],
                                ab_d[:, sb0 * DST_PAD:(sb0 + npk) * DST_PAD])
                    for j in range(npk):
                        sb = sb0 + j
                        for pl in range(nx):
                            if dr:
                                k = (sb * nx + pl) * 2
                                lhs = xs[:, k:k + 2, :]
                            else:
                                lhs = xs[:, sb * 128:(sb + 1) * 128]
                            for ci, (off, sz) in enumerate(CHUNKS):
                                if dr:
                                    rhs = cfull[:, 2 * sb:2 * sb + 2, off:off + sz]
                                else:
                                    rhs = cfull[:, sb * DST_PAD + off:
                                                sb * DST_PAD + off + sz]
                                nc.tensor.matmul(
                                    ps[ci][:], lhsT=lhs, rhs=rhs,
                                    start=(sb == 0 and pl == 0),
                                    stop=(sb == nsb - 1 and pl == nx - 1),
                                    perf_mode=pm)
                    sb0 += npk
                    if PIPELINE_TAIL and prev_za is not None:
                        for t in range(pk * tpp, min((pk + 1) * tpp, 10)):
                            emit_tail_tile(t, prev_za)
                za = wpool.tile([128, DST_PER_CORE], bf16, tag="za", name="za")
                for ci, (off, sz) in enumerate(CHUNKS):
                    nc.vector.tensor_mul(za[:, off:off + sz], ps[ci][:],
                                         dv[:, off:off + sz])
                if PIPELINE_TAIL:
                    prev_za = za
                else:
                    for t in range(10):
                        emit_tail_tile(t, za)
            if PIPELINE_TAIL and prev_za is not None:
                for t in range(10):
                    emit_tail_tile(t, prev_za)

    nc.compile()
    _nc_cache[key] = nc
    return nc


def _build_count_matrix(src, dst, rows):
    C = np.zeros((rows, N), np.float32)
    try:
        import scipy.sparse as sp
        ones = np.ones(src.shape[0], np.float32)
        M = sp.coo_matrix((ones, (src, dst)), shape=(rows, N)).tocsr()
        C[:] = M.toarray()
    except Exception:
        np.add.at(C, (src, dst), 1.0)
    C[np.arange(N), np.arange(N)] += 1.0
    return C


def _interleave_si(hi_blk, lo_blk):
    """[128p,128f] K-half planes -> SwInterleave 256-col layout:
    col 2*(127-f)+j holds plane_j[p, f]."""
    out = np.zeros((128, 256), np.float32)
    f = np.arange(128)
    out[:, 2 * (127 - f) + 0] = hi_blk
    out[:, 2 * (127 - f) + 1] = lo_blk
    return out


def prepare_in_maps(x, edge_index, W, b, gamma, beta, variant=None):
    variant = variant or VARIANT
    dr = variant in ("dr", "drsi")
    si = variant == "drsi"
    x = np.asarray(x, np.float32)
    W = np.asarray(W, np.float32)
    b = np.asarray(b, np.float32)
    gamma = np.asarray(gamma, np.float32)
    beta = np.asarray(beta, np.float32)
    src = np.asarray(edge_index[0], np.int64)
    dst = np.asarray(edge_index[1], np.int64)

    deg = np.bincount(dst, minlength=N).astype(np.float32) + 1.0
    dinv = (1.0 / np.sqrt(deg)).astype(np.float32)

    SRC_PAD = 40 * 256 if dr else 79 * 128
    C = _build_count_matrix(src, dst, SRC_PAD)
    xp = np.zeros((SRC_PAD, D), np.float32)
    xp[:N] = x * dinv[:, None]

    if dr:
        xhi = xp.astype(FP8).astype(np.float32)
        xlo = (xp - xhi).astype(FP8).astype(np.float32)
        xs_parts = []
        for sb in range(40):
            r0 = sb * 256
            for plane in (xhi, xlo):
                blk = plane[r0:r0 + 256].reshape(2, 128, 128)   # [i, p, f]
                if si:
                    xs_parts.append(_interleave_si(blk[0], blk[1]))
                else:
                    xs_parts.append(blk.transpose(1, 0, 2).reshape(128, 256))
        xs = np.ascontiguousarray(np.concatenate(xs_parts, 1)).astype(FP8)
    else:
        xs = np.ascontiguousarray(
            xp.reshape(79, 128, D).transpose(1, 0, 2).reshape(128, SRC_PAD)
        ).astype(BF16)

    wt = np.ascontiguousarray(W).astype(BF16)
    bb = np.ascontiguousarray(np.broadcast_to(b, (128, 128))).astype(np.float32)
    gb = np.ascontiguousarray(np.broadcast_to(gamma, (128, 128))).astype(np.float32)
    be = np.ascontiguousarray(np.broadcast_to(beta, (128, 128))).astype(np.float32)

    in_maps = []
    for c in range(NCORES):
        d0 = c * DST_PER_CORE
        Ac = np.zeros((SRC_PAD, DST_PAD), np.float32)
        Ac[:, :DST_PER_CORE] = C[:, d0:d0 + DST_PER_CORE]
        if dr:
            ab = np.ascontiguousarray(
                Ac.reshape(40, 2, 128, DST_PAD).transpose(2, 0, 1, 3)
                .reshape(128, 40 * 2 * DST_PAD)).astype(FP8)
        else:
            ab = np.ascontiguousarray(
                Ac.reshape(79, 128, DST_PAD).transpose(1, 0, 2)
                .reshape(128, 79 * DST_PAD)).astype(FP8)
        dvv = dinv[d0:d0 + DST_PER_CORE]
        dvb = np.ascontiguousarray(np.broadcast_to(dvv, (128, DST_PER_CORE)))
        in_maps.append({"xs": xs, "ab": ab, "wt": wt, "dv": dvb,
                        "bb": bb, "gb": gb, "be": be})
    return in_maps


def assemble_output(results):
    parts = []
    for c in range(NCORES):
        o = np.asarray(results[c]["out"], np.float32)
        parts.append(o[:DST_PER_CORE])
    return np.ascontiguousarray(np.concatenate(parts, axis=0))


def kernel(x, edge_index, W, b, gamma, beta):
    from concourse.bass_utils import run_bass_kernel_spmd

    nc = build_nc()
    in_maps = prepare_in_maps(x, edge_index, W, b, gamma, beta)
    res = run_bass_kernel_spmd(nc, in_maps, core_ids=list(range(NCORES)))
    return assemble_output(res.results)


if __name__ == "__main__":
    rng = np.random.default_rng(0)
    x = rng.normal(size=(N, D)).astype(np.float32)
    ei = rng.integers(0, N, size=(2, E))
    W = rng.normal(size=(D, D)).astype(np.float32) * 0.1
    b = np.zeros(D, np.float32)
    g = np.ones(D, np.float32)
    be = np.zeros(D, np.float32)
    out = kernel(x, ei, W, b, g, be)
    print(out.shape, out.dtype)
